# revision 4
# baseline (speedup 1.0000x reference)
"""Cross-attention kernel for Trainium2, distributed over 8 NeuronCores.

Sharding: batch x head parallel. Cores 0-3 handle batch 0, cores 4-7 batch 1.
Within a team of 4, core r handles heads 4r..4r+3 (channel slice 256r..256r+256)
and output columns 256r..256r+256 (column-parallel W_out).

Per core:
  - KV projection for its 256 k-channels + 256 v-channels (contraction over
    full D with host-pretransposed context/W_kv)
  - k AND q LayerNorm stats: bn_stats partials per row over the core's 256
    channels + one 32KB AllReduce within the team (mean, E[x^2] per tensor)
  - LN apply + transpose for k and q slices; gamma/beta fused into the
    PSUM->SBUF drain after the PE transpose
  - attention for its 4 heads, computed transposed (simT[j,i] = k.q) with
    softmax denominators from an appended ones-column in v (no max
    subtraction: |sim*scale| <= ~6 for this problem, exp stays in fp32 range);
    exp runs on j-tile PAIRS ([128,1024] per Activation op); denominators are
    inverted on DVE and partition-broadcast on the Pool engine (no DMA trip)
  - attention output transposed [256, NQ]; per-512-column-block AllGather
    within team -> [1024, 512]; each core projects the block against ITS
    256 columns of W_out only (no redundant work). Host assembles columns.
"""

import numpy as np

import concourse.bass as bass
import concourse.mybir as mybir
import concourse.tile as tile
from concourse import bacc
from concourse.bass_utils import run_bass_kernel_spmd
from concourse.masks import make_identity

B, NQ, NK, D, H, DH = 2, 2048, 2048, 1024, 16, 64
NCORES = 8
TEAM = 4
HPC = 4            # heads per core
DSL = HPC * DH     # 256: per-core channel slice
EPS = 1e-6
SCALE = DH ** -0.5
GROUPS = [[0, 1, 2, 3], [4, 5, 6, 7]]
FP32 = mybir.dt.float32
FP32R = mybir.dt.float32r
NT = NQ // 128     # 16 row tiles
KC = D // 128      # 8 contraction chunks
NBLK = 4           # 512-column blocks of NQ
BW = NQ // NBLK    # 512

_CACHE: dict = {}
MOCK_COLL = False  # replace collectives with local DMA (for TimelineSim)


def _bcast_ap(t, parts):
    ap = t.ap() if hasattr(t, "ap") and not isinstance(t, bass.AP) else t
    return bass.AP(tensor=ap.tensor, offset=ap.offset,
                   ap=[[0, parts]] + list(ap.ap))


def _build():
    nc = bacc.Bacc("TRN2", target_bir_lowering=False, debug=False,
                   num_devices=NCORES)
    x_s = nc.declare_dram_parameter("x_s", [NQ, DSL], FP32, isOutput=False)
    ctxT = nc.declare_dram_parameter("ctxT", [D, NK], FP32, isOutput=False)
    wkvT = nc.declare_dram_parameter("wkvT", [D, 2 * DSL], FP32, isOutput=False)
    woutT = nc.declare_dram_parameter("woutT", [D, DSL], FP32, isOutput=False)
    bout = nc.declare_dram_parameter("bout", [DSL], FP32, isOutput=False)
    gq_s = nc.declare_dram_parameter("gq_s", [DSL], FP32, isOutput=False)
    bq_s = nc.declare_dram_parameter("bq_s", [DSL], FP32, isOutput=False)
    gk_s = nc.declare_dram_parameter("gk_s", [DSL], FP32, isOutput=False)
    bk_s = nc.declare_dram_parameter("bk_s", [DSL], FP32, isOutput=False)
    y_col = nc.declare_dram_parameter("y_col", [NQ, DSL], FP32, isOutput=True)

    stats_dram = nc.dram_tensor("stats_dram", [128, 4 * NT], FP32)
    statsr_dram = nc.dram_tensor("statsr_dram", [128, 4 * NT], FP32)
    aoT_blk = [nc.dram_tensor(f"aoT_blk{i}", [DSL, BW], FP32)
               for i in range(NBLK)]
    agT_blk = [nc.dram_tensor(f"agT_blk{i}", [D, BW], FP32)
               for i in range(NBLK)]

    ctxT_r = ctxT.ap().rearrange("(k p) m -> p k m", p=128)    # [128, 8, NK]
    wkvT_r = wkvT.ap().rearrange("(k p) n -> p k n", p=128)    # [128, 8, 512]
    woutT_r = woutT.ap().rearrange("(k p) n -> p k n", p=128)  # [128, 8, 256]
    x_r = x_s.ap().rearrange("(t p) d -> p t d", p=128)        # [128, 16, 256]

    with tile.TileContext(nc) as tc:
        with (
            tc.tile_pool(name="singles", bufs=1) as singles,
            tc.tile_pool(name="ld", bufs=3) as ld,
            tc.tile_pool(name="work", bufs=3) as work,
            tc.tile_pool(name="psmm", bufs=2, space="PSUM") as psmm,
            tc.tile_pool(name="pssim", bufs=2, space="PSUM") as pssim,
            tc.tile_pool(name="psout", bufs=2, space="PSUM") as psout,
        ):
            # --- prologue: act-table warm-up + persistent sbuf loads ---
            dum = singles.tile([1, 2], FP32)
            nc.vector.memset(dum, 1.0)
            nc.scalar.activation(dum[:, 0:1], dum[:, 0:1],
                                 mybir.ActivationFunctionType.Sqrt)

            wkv_sb = singles.tile([128, KC, 2 * DSL], FP32R)
            nc.sync.dma_start(out=wkv_sb, in_=wkvT_r.bitcast(FP32R))

            def _col_ap(param, cb):
                ap = param.ap()
                return bass.AP(tensor=ap.tensor, offset=128 * cb,
                               ap=[[1, 128], [1, 1]])

            gqT = [singles.tile([128, 1], FP32, name=f"gqT{cb}") for cb in range(2)]
            bqT = [singles.tile([128, 1], FP32, name=f"bqT{cb}") for cb in range(2)]
            gkT = [singles.tile([128, 1], FP32, name=f"gkT{cb}") for cb in range(2)]
            bkT = [singles.tile([128, 1], FP32, name=f"bkT{cb}") for cb in range(2)]
            for cb in range(2):
                nc.gpsimd.dma_start(out=gqT[cb], in_=_col_ap(gq_s, cb))
                nc.gpsimd.dma_start(out=bqT[cb], in_=_col_ap(bq_s, cb))
                nc.gpsimd.dma_start(out=gkT[cb], in_=_col_ap(gk_s, cb))
                nc.gpsimd.dma_start(out=bkT[cb], in_=_col_ap(bk_s, cb))
            bout_b = singles.tile([128, DSL], FP32)
            nc.gpsimd.dma_start(out=bout_b, in_=_bcast_ap(bout, 128))

            x_nat = singles.tile([128, NT, DSL], FP32)
            nc.gpsimd.dma_start(out=x_nat, in_=x_r)

            ident = singles.tile([128, 128], FP32)
            make_identity(nc, ident)
            eps_sb = singles.tile([128, 1], FP32)
            nc.vector.memset(eps_sb, EPS)

            k_nat = singles.tile([128, NT, DSL], FP32)
            vh = singles.tile([128, NT, HPC, DH + 2], FP32R)
            nc.vector.memset(vh[:, :, :, DH:DH + 1].bitcast(FP32), 1.0)
            nc.vector.memset(vh[:, :, :, DH + 1:DH + 2].bitcast(FP32), 0.0)
            qT_sb = [singles.tile([128, NT, 128], FP32R, tag=f"qT{cb}",
                                  name=f"qT{cb}") for cb in range(2)]
            kT_sb = [singles.tile([128, NT, 128], FP32R, tag=f"kT{cb}",
                                  name=f"kT{cb}") for cb in range(2)]
            aoT_sb = [singles.tile([128, NQ], FP32, tag=f"aoT{cb}",
                                   name=f"aoT{cb}") for cb in range(2)]
            mvk = singles.tile([128, NT, 2], FP32)
            mvq = singles.tile([128, NT, 2], FP32)
            stats_sb = singles.tile([128, 4, NT], FP32)
            statsr_sb = singles.tile([128, 4, NT], FP32)
            mean_k = singles.tile([128, NT], FP32)
            rstd_k = singles.tile([128, NT], FP32)
            mean_q = singles.tile([128, NT], FP32)
            rstd_q = singles.tile([128, NT], FP32)

            # --- stage A: kv-proj + k/q partial LN stats ---
            for t in range(NT):
                ctx_sb = ld.tile([128, KC, 128], FP32R, tag="ctx")
                nc.sync.dma_start(out=ctx_sb,
                                  in_=ctxT_r[:, :, 128 * t:128 * (t + 1)]
                                  .bitcast(FP32R))
                kv_ps = psmm.tile([128, 2 * DSL], FP32, tag="mm512")
                for kk in range(KC):
                    nc.tensor.matmul(kv_ps, lhsT=ctx_sb[:, kk, :],
                                     rhs=wkv_sb[:, kk, :],
                                     start=(kk == 0), stop=(kk == KC - 1))
                nc.vector.tensor_copy(k_nat[:, t, :], kv_ps[:, 0:DSL])
                nc.vector.tensor_copy(
                    vh[:, t, :, 0:DH],
                    kv_ps[:, DSL:2 * DSL].rearrange("p (h d) -> p h d", h=HPC))
                bnk = work.tile([128, 6], FP32, tag="bn6")
                nc.vector.bn_stats(out=bnk, in_=k_nat[:, t, :])
                nc.vector.bn_aggr(out=mvk[:, t, :], in_=bnk)
                bnq = work.tile([128, 6], FP32, tag="bn6")
                nc.vector.bn_stats(out=bnq, in_=x_nat[:, t, :])
                nc.vector.bn_aggr(out=mvq[:, t, :], in_=bnq)

            # wout only needed in stage F; queue its load behind x
            wout_sb = singles.tile([128, KC, DSL], FP32R)
            nc.gpsimd.dma_start(out=wout_sb, in_=woutT_r.bitcast(FP32R))

            # --- stage B: AllReduce (mean, E[x^2]) for k and q ---
            tmp16 = work.tile([128, NT], FP32, tag="t16")
            nc.vector.tensor_copy(stats_sb[:, 0, :], mvk[:, :, 0])
            nc.vector.tensor_mul(tmp16, mvk[:, :, 0], mvk[:, :, 0])
            nc.vector.tensor_add(stats_sb[:, 1, :], mvk[:, :, 1], tmp16)
            tmq16 = work.tile([128, NT], FP32, tag="t16")
            nc.vector.tensor_copy(stats_sb[:, 2, :], mvq[:, :, 0])
            nc.vector.tensor_mul(tmq16, mvq[:, :, 0], mvq[:, :, 0])
            nc.vector.tensor_add(stats_sb[:, 3, :], mvq[:, :, 1], tmq16)
            nc.sync.dma_start(out=stats_dram[:, :],
                              in_=stats_sb.rearrange("p s t -> p (s t)"))
            if MOCK_COLL:
                nc.sync.dma_start(out=statsr_dram[:, :], in_=stats_dram[:, :])
            else:
                nc.gpsimd.collective_compute(
                    "AllReduce", mybir.AluOpType.add, replica_groups=GROUPS,
                    ins=[stats_dram.ap().opt()], outs=[statsr_dram.ap().opt()])
            nc.sync.dma_start(out=statsr_sb.rearrange("p s t -> p (s t)"),
                              in_=statsr_dram[:, :])

            def _finish_stats(row, mean_out, rstd_out):
                nc.vector.tensor_scalar_mul(mean_out,
                                            in0=statsr_sb[:, row, :],
                                            scalar1=1.0 / TEAM)
                e2 = work.tile([128, NT], FP32, tag="t16")
                nc.vector.tensor_scalar_mul(e2, in0=statsr_sb[:, row + 1, :],
                                            scalar1=1.0 / TEAM)
                m2 = work.tile([128, NT], FP32, tag="t16")
                nc.vector.tensor_mul(m2, mean_out, mean_out)
                nc.vector.tensor_sub(e2, e2, m2)
                nc.scalar.activation(e2, e2,
                                     mybir.ActivationFunctionType.Sqrt,
                                     bias=eps_sb)
                nc.vector.reciprocal(rstd_out, e2)

            _finish_stats(0, mean_k, rstd_k)
            _finish_stats(2, mean_q, rstd_q)
            # warm the exp table while stage D runs
            nc.scalar.activation(dum[:, 1:2], dum[:, 1:2],
                                 mybir.ActivationFunctionType.Exp)

            # --- stage D: LN apply + transpose for k and q ---
            for t in range(NT):
                kn = work.tile([128, DSL], FP32, tag="kn")
                nc.vector.tensor_scalar(out=kn, in0=k_nat[:, t, :],
                                        scalar1=mean_k[:, t:t + 1],
                                        scalar2=rstd_k[:, t:t + 1],
                                        op0=mybir.AluOpType.subtract,
                                        op1=mybir.AluOpType.mult)
                qn = work.tile([128, DSL], FP32, tag="qn")
                nc.vector.tensor_scalar(out=qn, in0=x_nat[:, t, :],
                                        scalar1=mean_q[:, t:t + 1],
                                        scalar2=rstd_q[:, t:t + 1],
                                        op0=mybir.AluOpType.subtract,
                                        op1=mybir.AluOpType.mult)
                tp = psmm.tile([128, 512], FP32, tag="mm512")
                for cb in range(2):
                    nc.tensor.transpose(tp[:, 128 * cb:128 * (cb + 1)],
                                        kn[:, 128 * cb:128 * (cb + 1)], ident)
                    nc.tensor.transpose(tp[:, 256 + 128 * cb:384 + 128 * cb],
                                        qn[:, 128 * cb:128 * (cb + 1)], ident)
                for cb in range(2):
                    nc.vector.tensor_scalar(out=kT_sb[cb][:, t, :],
                                            in0=tp[:, 128 * cb:128 * (cb + 1)],
                                            scalar1=gkT[cb], scalar2=bkT[cb],
                                            op0=mybir.AluOpType.mult,
                                            op1=mybir.AluOpType.add)
                    nc.vector.tensor_scalar(out=qT_sb[cb][:, t, :],
                                            in0=tp[:, 256 + 128 * cb:
                                                   384 + 128 * cb],
                                            scalar1=gqT[cb], scalar2=bqT[cb],
                                            op0=mybir.AluOpType.mult,
                                            op1=mybir.AluOpType.add)

            # --- stage F: attention -> per-block AllGather -> column-sharded
            # out-projection, pipelined over 512-column blocks of NQ ---
            for iblk in range(NBLK):
                for h in range(HPC):
                    cb, hh = h // 2, h % 2
                    khT = kT_sb[cb][64 * hh:64 * (hh + 1), :, :]
                    qhT = qT_sb[cb][64 * hh:64 * (hh + 1), :, :]
                    oT_ps = psout.tile([DH + 2, BW], FP32, tag="oT")
                    for jp in range(NT // 2):
                        s2 = pssim.tile([128, 2, BW], FP32, tag="sim")
                        nc.tensor.matmul(s2[:, 0, :], lhsT=khT[:, 2 * jp, :],
                                         rhs=qhT[:, 4 * iblk:4 * (iblk + 1), :],
                                         start=True, stop=True)
                        nc.tensor.matmul(s2[:, 1, :], lhsT=khT[:, 2 * jp + 1, :],
                                         rhs=qhT[:, 4 * iblk:4 * (iblk + 1), :],
                                         start=True, stop=True)
                        e2t = work.tile([128, 2, BW], FP32R, tag="exp", bufs=3)
                        nc.scalar.activation(e2t, s2,
                                             mybir.ActivationFunctionType.Exp,
                                             scale=SCALE)
                        nc.tensor.matmul(oT_ps, lhsT=vh[:, 2 * jp, h, :],
                                         rhs=e2t[:, 0, :],
                                         start=(jp == 0), stop=False)
                        nc.tensor.matmul(oT_ps, lhsT=vh[:, 2 * jp + 1, h, :],
                                         rhs=e2t[:, 1, :],
                                         start=False, stop=(jp == NT // 2 - 1))
                    # normalize: row DH of oT_ps holds the softmax denominators
                    rcp = work.tile([1, BW], FP32, tag="rcp", bufs=2)
                    nc.vector.reciprocal(rcp, oT_ps[DH:DH + 1, :])
                    den = work.tile([DH, BW], FP32, tag="den", bufs=2)
                    nc.gpsimd.partition_broadcast(den, rcp)
                    nc.vector.tensor_mul(
                        aoT_sb[cb][64 * hh:64 * (hh + 1),
                                   BW * iblk:BW * (iblk + 1)],
                        oT_ps[0:DH, :], den)
                # gather this column block and project it while later
                # blocks are still in flight
                for cb in range(2):
                    nc.sync.dma_start(
                        out=aoT_blk[iblk][128 * cb:128 * (cb + 1), :],
                        in_=aoT_sb[cb][:, BW * iblk:BW * (iblk + 1)])
                if MOCK_COLL:
                    nc.sync.dma_start(out=agT_blk[iblk][0:DSL, :],
                                      in_=aoT_blk[iblk][:, :])
                else:
                    nc.gpsimd.collective_compute(
                        "AllGather", mybir.AluOpType.bypass,
                        replica_groups=GROUPS,
                        ins=[aoT_blk[iblk].ap().opt()],
                        outs=[agT_blk[iblk].ap().opt()])
                ag_r = agT_blk[iblk].ap().rearrange("(k p) n -> p k n", p=128)
                for sub in range(4):
                    nt = 4 * iblk + sub
                    ag_sb = ld.tile([128, KC, 128], FP32R, tag="ctx",
                                    name="ag_sb")
                    nc.sync.dma_start(
                        out=ag_sb,
                        in_=ag_r[:, :, 128 * sub:128 * (sub + 1)].bitcast(FP32R))
                    y_ps = psmm.tile([128, 512], FP32, tag="mm512",
                                     name="y_ps")
                    for kk in range(KC):
                        nc.tensor.matmul(y_ps[:, 0:DSL], lhsT=ag_sb[:, kk, :],
                                         rhs=wout_sb[:, kk, :],
                                         start=(kk == 0), stop=(kk == KC - 1))
                    y_sb = work.tile([128, DSL], FP32, tag="y", bufs=2)
                    nc.vector.tensor_add(y_sb, y_ps[:, 0:DSL], bout_b)
                    nc.sync.dma_start(out=y_col[128 * nt:128 * (nt + 1), :],
                                      in_=y_sb)

    nc.finalize()
    return nc


def kernel(x, context, gq, bq, gk, bk, W_kv, W_out, b_out):
    x = np.asarray(x, dtype=np.float32)
    context = np.asarray(context, dtype=np.float32)
    gq = np.asarray(gq, dtype=np.float32)
    bq = np.asarray(bq, dtype=np.float32)
    gk = np.asarray(gk, dtype=np.float32)
    bk = np.asarray(bk, dtype=np.float32)
    W_kv = np.asarray(W_kv, dtype=np.float32)
    W_out = np.asarray(W_out, dtype=np.float32)
    b_out = np.asarray(b_out, dtype=np.float32)

    if "nc" not in _CACHE:
        _CACHE["nc"] = _build()
    nc = _CACHE["nc"]

    Wk, Wv = W_kv[:D], W_kv[D:]
    in_maps = []
    for c in range(NCORES):
        b, r = c // TEAM, c % TEAM
        sl = slice(DSL * r, DSL * (r + 1))
        wkvT_c = np.ascontiguousarray(
            np.concatenate([Wk[sl], Wv[sl]], axis=0).T)
        in_maps.append({
            "x_s": np.ascontiguousarray(x[b][:, sl]),
            "ctxT": np.ascontiguousarray(context[b].T),
            "wkvT": wkvT_c,
            "woutT": np.ascontiguousarray(W_out[sl].T),
            "bout": np.ascontiguousarray(b_out[sl]),
            "gq_s": np.ascontiguousarray(gq[sl]),
            "bq_s": np.ascontiguousarray(bq[sl]),
            "gk_s": np.ascontiguousarray(gk[sl]),
            "bk_s": np.ascontiguousarray(bk[sl]),
        })

    _CACHE["in_maps"] = in_maps
    try:
        res = run_bass_kernel_spmd(nc, in_maps, list(range(NCORES))).results
    except Exception:
        # transient runtime failures (device wedged from a prior run) --
        # one retry typically succeeds
        res = run_bass_kernel_spmd(nc, in_maps, list(range(NCORES))).results
    y = np.empty((B, NQ, D), dtype=np.float32)
    for c in range(NCORES):
        b, r = c // TEAM, c % TEAM
        y[b, :, DSL * r:DSL * (r + 1)] = res[c]["y_col"]
    return y


# revision 5
# speedup vs baseline: 1.0424x; 1.0424x over previous
"""Cross-attention kernel for Trainium2, distributed over 8 NeuronCores.

Sharding: batch x head parallel. Cores 0-3 handle batch 0, cores 4-7 batch 1.
Within a team of 4, core r handles heads 4r..4r+3 (channel slice 256r..256r+256)
and output columns 256r..256r+256 (column-parallel W_out).

Per core:
  - KV projection for its 256 k-channels + 256 v-channels (contraction over
    full D with host-pretransposed context/W_kv)
  - k AND q LayerNorm stats: bn_stats partials per row over the core's 256
    channels, AllReduced within the team in TWO rounds (k tiles 0-7 + all q
    tiles at mid-stage-A; k tiles 8-15 at the end) so LN apply / transpose /
    attention start while the tail of the KV projection still streams
  - rstd computed as exp(-0.5*ln(var+eps)): Ln and Exp share one activation
    table set, so the whole kernel needs a single act-table load (warmed at
    t=0), instead of sqrt<->exp table thrash
  - attention for its 4 heads, computed transposed (simT[j,i] = k.q) with
    softmax denominators from an appended ones-column in v (no max
    subtraction: |sim*scale| <= ~6 for this problem, exp stays in fp32 range);
    exp runs on j-tile PAIRS ([128,1024] per Activation op); denominators are
    inverted on DVE and partition-broadcast on the Pool engine (no DMA trip)
  - attention output transposed [256, NQ]; per-512-column-block AllGather
    within team -> [1024, 512]; each core projects the block against ITS
    256 columns of W_out only. The projection of block b is EMITTED after
    block b+1's attention so the gather latency hides under compute.
    Host assembles the column slices.
"""

import numpy as np

import concourse.bass as bass
import concourse.mybir as mybir
import concourse.tile as tile
from concourse import bacc
from concourse.bass_utils import run_bass_kernel_spmd
from concourse.masks import make_identity

B, NQ, NK, D, H, DH = 2, 2048, 2048, 1024, 16, 64
NCORES = 8
TEAM = 4
HPC = 4            # heads per core
DSL = HPC * DH     # 256: per-core channel slice
EPS = 1e-6
SCALE = DH ** -0.5
GROUPS = [[0, 1, 2, 3], [4, 5, 6, 7]]
FP32 = mybir.dt.float32
FP32R = mybir.dt.float32r
NT = NQ // 128     # 16 row tiles
NH = NT // 2       # 8: tiles per stats round
KC = D // 128      # 8 contraction chunks
NBLK = 4           # 512-column blocks of NQ
BW = NQ // NBLK    # 512

_CACHE: dict = {}
MOCK_COLL = False  # replace collectives with local DMA (for TimelineSim)


def _bcast_ap(t, parts):
    ap = t.ap() if hasattr(t, "ap") and not isinstance(t, bass.AP) else t
    return bass.AP(tensor=ap.tensor, offset=ap.offset,
                   ap=[[0, parts]] + list(ap.ap))


def _build():
    nc = bacc.Bacc("TRN2", target_bir_lowering=False, debug=False,
                   num_devices=NCORES)
    x_s = nc.declare_dram_parameter("x_s", [NQ, DSL], FP32, isOutput=False)
    ctxT = nc.declare_dram_parameter("ctxT", [D, NK], FP32, isOutput=False)
    wkvT = nc.declare_dram_parameter("wkvT", [D, 2 * DSL], FP32, isOutput=False)
    woutT = nc.declare_dram_parameter("woutT", [D, DSL], FP32, isOutput=False)
    bout = nc.declare_dram_parameter("bout", [DSL], FP32, isOutput=False)
    gq_s = nc.declare_dram_parameter("gq_s", [DSL], FP32, isOutput=False)
    bq_s = nc.declare_dram_parameter("bq_s", [DSL], FP32, isOutput=False)
    gk_s = nc.declare_dram_parameter("gk_s", [DSL], FP32, isOutput=False)
    bk_s = nc.declare_dram_parameter("bk_s", [DSL], FP32, isOutput=False)
    y_col = nc.declare_dram_parameter("y_col", [NQ, DSL], FP32, isOutput=True)

    # round 1: k tiles 0-7 (mean, E[x^2]) + q tiles 0-15; round 2: k tiles 8-15
    stats1_dram = nc.dram_tensor("stats1_dram", [128, 2 * NH + 2 * NT], FP32)
    statsr1_dram = nc.dram_tensor("statsr1_dram", [128, 2 * NH + 2 * NT], FP32)
    stats2_dram = nc.dram_tensor("stats2_dram", [128, 2 * NH], FP32)
    statsr2_dram = nc.dram_tensor("statsr2_dram", [128, 2 * NH], FP32)
    aoT_blk = [nc.dram_tensor(f"aoT_blk{i}", [DSL, BW], FP32)
               for i in range(NBLK)]
    agT_blk = [nc.dram_tensor(f"agT_blk{i}", [D, BW], FP32)
               for i in range(NBLK)]

    ctxT_r = ctxT.ap().rearrange("(k p) m -> p k m", p=128)    # [128, 8, NK]
    wkvT_r = wkvT.ap().rearrange("(k p) n -> p k n", p=128)    # [128, 8, 512]
    woutT_r = woutT.ap().rearrange("(k p) n -> p k n", p=128)  # [128, 8, 256]
    x_r = x_s.ap().rearrange("(t p) d -> p t d", p=128)        # [128, 16, 256]

    with tile.TileContext(nc) as tc:
        with (
            tc.tile_pool(name="singles", bufs=1) as singles,
            tc.tile_pool(name="ld", bufs=3) as ld,
            tc.tile_pool(name="work", bufs=3) as work,
            tc.tile_pool(name="psmm", bufs=2, space="PSUM") as psmm,
            tc.tile_pool(name="pssim", bufs=2, space="PSUM") as pssim,
            tc.tile_pool(name="psout", bufs=2, space="PSUM") as psout,
        ):
            # --- prologue: act-table warm-up + persistent sbuf loads ---
            dum = singles.tile([1, 2], FP32)
            nc.vector.memset(dum, 1.0)
            nc.scalar.activation(dum[:, 0:1], dum[:, 0:1],
                                 mybir.ActivationFunctionType.Ln)
            nc.scalar.activation(dum[:, 1:2], dum[:, 1:2],
                                 mybir.ActivationFunctionType.Exp)

            x_nat = singles.tile([128, NT, DSL], FP32)
            nc.gpsimd.dma_start(out=x_nat, in_=x_r)
            wkv_sb = singles.tile([128, KC, 2 * DSL], FP32R)
            nc.sync.dma_start(out=wkv_sb, in_=wkvT_r.bitcast(FP32R))

            def _col_ap(param, cb):
                ap = param.ap()
                return bass.AP(tensor=ap.tensor, offset=128 * cb,
                               ap=[[1, 128], [1, 1]])

            gqT = [singles.tile([128, 1], FP32, name=f"gqT{cb}") for cb in range(2)]
            bqT = [singles.tile([128, 1], FP32, name=f"bqT{cb}") for cb in range(2)]
            gkT = [singles.tile([128, 1], FP32, name=f"gkT{cb}") for cb in range(2)]
            bkT = [singles.tile([128, 1], FP32, name=f"bkT{cb}") for cb in range(2)]
            for cb in range(2):
                nc.gpsimd.dma_start(out=gqT[cb], in_=_col_ap(gq_s, cb))
                nc.gpsimd.dma_start(out=bqT[cb], in_=_col_ap(bq_s, cb))
                nc.gpsimd.dma_start(out=gkT[cb], in_=_col_ap(gk_s, cb))
                nc.gpsimd.dma_start(out=bkT[cb], in_=_col_ap(bk_s, cb))
            bout_b = singles.tile([128, DSL], FP32)
            nc.gpsimd.dma_start(out=bout_b, in_=_bcast_ap(bout, 128))

            ident = singles.tile([128, 128], FP32)
            make_identity(nc, ident)
            eps_sb = singles.tile([128, 1], FP32)
            nc.vector.memset(eps_sb, EPS)

            k_nat = singles.tile([128, NT, DSL], FP32)
            vh = singles.tile([128, NT, HPC, DH + 2], FP32R)
            nc.vector.memset(vh[:, :, :, DH:DH + 1].bitcast(FP32), 1.0)
            nc.vector.memset(vh[:, :, :, DH + 1:DH + 2].bitcast(FP32), 0.0)
            qT_sb = [singles.tile([128, NT, 128], FP32R, tag=f"qT{cb}",
                                  name=f"qT{cb}") for cb in range(2)]
            kT_sb = [singles.tile([128, NT, 128], FP32R, tag=f"kT{cb}",
                                  name=f"kT{cb}") for cb in range(2)]
            aoT_sb = [singles.tile([128, NQ], FP32, tag=f"aoT{cb}",
                                   name=f"aoT{cb}") for cb in range(2)]
            mvk = singles.tile([128, NT, 2], FP32)
            mvq = singles.tile([128, NT, 2], FP32)
            stats1_sb = singles.tile([128, 2 * NH + 2 * NT], FP32)
            statsr1_sb = singles.tile([128, 2 * NH + 2 * NT], FP32)
            stats2_sb = singles.tile([128, 2 * NH], FP32)
            statsr2_sb = singles.tile([128, 2 * NH], FP32)
            mean_k = singles.tile([128, NT], FP32)
            rstd_k = singles.tile([128, NT], FP32)
            mean_q = singles.tile([128, NT], FP32)
            rstd_q = singles.tile([128, NT], FP32)

            # q partial stats: only need x, start immediately
            for t in range(NT):
                bnq = work.tile([128, 6], FP32, tag="bn6")
                nc.vector.bn_stats(out=bnq, in_=x_nat[:, t, :])
                nc.vector.bn_aggr(out=mvq[:, t, :], in_=bnq)

            def _pack(dst, mv, t0, t1):
                # dst[:, 0:n] = partial mean; dst[:, n:2n] = partial E[x^2]
                n = t1 - t0
                tmp = work.tile([128, n], FP32, tag="t16")
                nc.vector.tensor_copy(dst[:, 0:n], mv[:, t0:t1, 0])
                nc.vector.tensor_mul(tmp, mv[:, t0:t1, 0], mv[:, t0:t1, 0])
                nc.vector.tensor_add(dst[:, n:2 * n], mv[:, t0:t1, 1], tmp)

            def _finish(src, off, n, mean_out, rstd_out, t0):
                nc.vector.tensor_scalar_mul(mean_out[:, t0:t0 + n],
                                            in0=src[:, off:off + n],
                                            scalar1=1.0 / TEAM)
                e2 = work.tile([128, n], FP32, tag="t16")
                nc.vector.tensor_scalar_mul(e2, in0=src[:, off + n:off + 2 * n],
                                            scalar1=1.0 / TEAM)
                m2 = work.tile([128, n], FP32, tag="t16")
                nc.vector.tensor_mul(m2, mean_out[:, t0:t0 + n],
                                     mean_out[:, t0:t0 + n])
                nc.vector.tensor_sub(e2, e2, m2)
                # rstd = exp(-0.5 * ln(var + eps)); Ln+Exp share a table set
                nc.scalar.activation(e2, e2, mybir.ActivationFunctionType.Ln,
                                     bias=eps_sb)
                nc.scalar.activation(rstd_out[:, t0:t0 + n], e2,
                                     mybir.ActivationFunctionType.Exp,
                                     scale=-0.5)

            def _emit_d(src_nat, mean, rstd, gT, bT, dstT, t):
                n_ = work.tile([128, DSL], FP32, tag="dnorm")
                nc.vector.tensor_scalar(out=n_, in0=src_nat[:, t, :],
                                        scalar1=mean[:, t:t + 1],
                                        scalar2=rstd[:, t:t + 1],
                                        op0=mybir.AluOpType.subtract,
                                        op1=mybir.AluOpType.mult)
                tp = psmm.tile([128, 512], FP32, tag="mm512")
                for cb in range(2):
                    nc.tensor.transpose(tp[:, 128 * cb:128 * (cb + 1)],
                                        n_[:, 128 * cb:128 * (cb + 1)], ident)
                for cb in range(2):
                    nc.vector.tensor_scalar(out=dstT[cb][:, t, :],
                                            in0=tp[:, 128 * cb:128 * (cb + 1)],
                                            scalar1=gT[cb], scalar2=bT[cb],
                                            op0=mybir.AluOpType.mult,
                                            op1=mybir.AluOpType.add)

            # --- stage A: kv-proj + k partial stats; stats round 1 at t=7 ---
            for t in range(NT):
                ctx_sb = ld.tile([128, KC, 128], FP32R, tag="ctx")
                nc.sync.dma_start(out=ctx_sb,
                                  in_=ctxT_r[:, :, 128 * t:128 * (t + 1)]
                                  .bitcast(FP32R))
                kv_ps = psmm.tile([128, 2 * DSL], FP32, tag="mm512")
                for kk in range(KC):
                    nc.tensor.matmul(kv_ps, lhsT=ctx_sb[:, kk, :],
                                     rhs=wkv_sb[:, kk, :],
                                     start=(kk == 0), stop=(kk == KC - 1))
                nc.vector.tensor_copy(k_nat[:, t, :], kv_ps[:, 0:DSL])
                nc.vector.tensor_copy(
                    vh[:, t, :, 0:DH],
                    kv_ps[:, DSL:2 * DSL].rearrange("p (h d) -> p h d", h=HPC))
                bnk = work.tile([128, 6], FP32, tag="bn6")
                nc.vector.bn_stats(out=bnk, in_=k_nat[:, t, :])
                nc.vector.bn_aggr(out=mvk[:, t, :], in_=bnk)

                if t == NH - 1:
                    # stats round 1: k tiles 0-7 + all q tiles
                    _pack(stats1_sb[:, 0:2 * NH], mvk, 0, NH)
                    _pack(stats1_sb[:, 2 * NH:], mvq, 0, NT)
                    nc.sync.dma_start(out=stats1_dram[:, :], in_=stats1_sb)
                    if MOCK_COLL:
                        nc.sync.dma_start(out=statsr1_dram[:, :],
                                          in_=stats1_dram[:, :])
                    else:
                        nc.gpsimd.collective_compute(
                            "AllReduce", mybir.AluOpType.add,
                            replica_groups=GROUPS,
                            ins=[stats1_dram.ap().opt()],
                            outs=[statsr1_dram.ap().opt()])
                    nc.sync.dma_start(out=statsr1_sb, in_=statsr1_dram[:, :])
                    _finish(statsr1_sb, 0, NH, mean_k, rstd_k, 0)
                    _finish(statsr1_sb, 2 * NH, NT, mean_q, rstd_q, 0)

            # wout only needed in stage F; queue its load behind x
            wout_sb = singles.tile([128, KC, DSL], FP32R)
            nc.gpsimd.dma_start(out=wout_sb, in_=woutT_r.bitcast(FP32R))

            # stats round 2: k tiles 8-15
            _pack(stats2_sb, mvk, NH, NT)
            nc.sync.dma_start(out=stats2_dram[:, :], in_=stats2_sb)
            if MOCK_COLL:
                nc.sync.dma_start(out=statsr2_dram[:, :], in_=stats2_dram[:, :])
            else:
                nc.gpsimd.collective_compute(
                    "AllReduce", mybir.AluOpType.add, replica_groups=GROUPS,
                    ins=[stats2_dram.ap().opt()],
                    outs=[statsr2_dram.ap().opt()])
            nc.sync.dma_start(out=statsr2_sb, in_=statsr2_dram[:, :])
            _finish(statsr2_sb, 0, NH, mean_k, rstd_k, NH)

            # --- stage D: LN apply + transpose, ordered to unblock attention:
            # q tiles 0-3 (block 0), k tiles 0-7, q 4-15, then k 8-15 ---
            for t in range(4):
                _emit_d(x_nat, mean_q, rstd_q, gqT, bqT, qT_sb, t)
            for t in range(NH):
                _emit_d(k_nat, mean_k, rstd_k, gkT, bkT, kT_sb, t)
            for t in range(4, NT):
                _emit_d(x_nat, mean_q, rstd_q, gqT, bqT, qT_sb, t)
            for t in range(NH, NT):
                _emit_d(k_nat, mean_k, rstd_k, gkT, bkT, kT_sb, t)

            # --- stage F: attention -> per-block AllGather -> column-sharded
            # out-projection. Projection of block b is emitted after block
            # b+1's attention so the gather hides under compute. ---
            def _emit_proj(iblk):
                ag_r = agT_blk[iblk].ap().rearrange("(k p) n -> p k n", p=128)
                for sub in range(4):
                    nt = 4 * iblk + sub
                    ag_sb = ld.tile([128, KC, 128], FP32R, tag="ctx",
                                    name="ag_sb")
                    nc.sync.dma_start(
                        out=ag_sb,
                        in_=ag_r[:, :, 128 * sub:128 * (sub + 1)]
                        .bitcast(FP32R))
                    y_ps = psmm.tile([128, 512], FP32, tag="mm512",
                                     name="y_ps")
                    for kk in range(KC):
                        nc.tensor.matmul(y_ps[:, 0:DSL], lhsT=ag_sb[:, kk, :],
                                         rhs=wout_sb[:, kk, :],
                                         start=(kk == 0), stop=(kk == KC - 1))
                    y_sb = work.tile([128, DSL], FP32, tag="y", bufs=2)
                    nc.vector.tensor_add(y_sb, y_ps[:, 0:DSL], bout_b)
                    nc.sync.dma_start(out=y_col[128 * nt:128 * (nt + 1), :],
                                      in_=y_sb)

            for iblk in range(NBLK):
                for h in range(HPC):
                    cb, hh = h // 2, h % 2
                    khT = kT_sb[cb][64 * hh:64 * (hh + 1), :, :]
                    qhT = qT_sb[cb][64 * hh:64 * (hh + 1), :, :]
                    oT_ps = psout.tile([DH + 2, BW], FP32, tag="oT")
                    for jp in range(NT // 2):
                        s2 = pssim.tile([128, 2, BW], FP32, tag="sim")
                        nc.tensor.matmul(s2[:, 0, :], lhsT=khT[:, 2 * jp, :],
                                         rhs=qhT[:, 4 * iblk:4 * (iblk + 1), :],
                                         start=True, stop=True)
                        nc.tensor.matmul(s2[:, 1, :], lhsT=khT[:, 2 * jp + 1, :],
                                         rhs=qhT[:, 4 * iblk:4 * (iblk + 1), :],
                                         start=True, stop=True)
                        e2t = work.tile([128, 2, BW], FP32R, tag="exp", bufs=3)
                        nc.scalar.activation(e2t, s2,
                                             mybir.ActivationFunctionType.Exp,
                                             scale=SCALE)
                        nc.tensor.matmul(oT_ps, lhsT=vh[:, 2 * jp, h, :],
                                         rhs=e2t[:, 0, :],
                                         start=(jp == 0), stop=False)
                        nc.tensor.matmul(oT_ps, lhsT=vh[:, 2 * jp + 1, h, :],
                                         rhs=e2t[:, 1, :],
                                         start=False, stop=(jp == NT // 2 - 1))
                    # normalize: row DH of oT_ps holds the softmax denominators
                    rcp = work.tile([1, BW], FP32, tag="rcp", bufs=2)
                    nc.vector.reciprocal(rcp, oT_ps[DH:DH + 1, :])
                    den = work.tile([DH, BW], FP32, tag="den", bufs=2)
                    nc.gpsimd.partition_broadcast(den, rcp)
                    nc.vector.tensor_mul(
                        aoT_sb[cb][64 * hh:64 * (hh + 1),
                                   BW * iblk:BW * (iblk + 1)],
                        oT_ps[0:DH, :], den)
                # gather this column block; its projection is emitted after
                # the NEXT block's attention
                for cb in range(2):
                    nc.sync.dma_start(
                        out=aoT_blk[iblk][128 * cb:128 * (cb + 1), :],
                        in_=aoT_sb[cb][:, BW * iblk:BW * (iblk + 1)])
                if MOCK_COLL:
                    nc.sync.dma_start(out=agT_blk[iblk][0:DSL, :],
                                      in_=aoT_blk[iblk][:, :])
                else:
                    nc.gpsimd.collective_compute(
                        "AllGather", mybir.AluOpType.bypass,
                        replica_groups=GROUPS,
                        ins=[aoT_blk[iblk].ap().opt()],
                        outs=[agT_blk[iblk].ap().opt()])
                if iblk > 0:
                    _emit_proj(iblk - 1)
            _emit_proj(NBLK - 1)

    nc.finalize()
    return nc


def kernel(x, context, gq, bq, gk, bk, W_kv, W_out, b_out):
    x = np.asarray(x, dtype=np.float32)
    context = np.asarray(context, dtype=np.float32)
    gq = np.asarray(gq, dtype=np.float32)
    bq = np.asarray(bq, dtype=np.float32)
    gk = np.asarray(gk, dtype=np.float32)
    bk = np.asarray(bk, dtype=np.float32)
    W_kv = np.asarray(W_kv, dtype=np.float32)
    W_out = np.asarray(W_out, dtype=np.float32)
    b_out = np.asarray(b_out, dtype=np.float32)

    if "nc" not in _CACHE:
        _CACHE["nc"] = _build()
    nc = _CACHE["nc"]

    Wk, Wv = W_kv[:D], W_kv[D:]
    in_maps = []
    for c in range(NCORES):
        b, r = c // TEAM, c % TEAM
        sl = slice(DSL * r, DSL * (r + 1))
        wkvT_c = np.ascontiguousarray(
            np.concatenate([Wk[sl], Wv[sl]], axis=0).T)
        in_maps.append({
            "x_s": np.ascontiguousarray(x[b][:, sl]),
            "ctxT": np.ascontiguousarray(context[b].T),
            "wkvT": wkvT_c,
            "woutT": np.ascontiguousarray(W_out[sl].T),
            "bout": np.ascontiguousarray(b_out[sl]),
            "gq_s": np.ascontiguousarray(gq[sl]),
            "bq_s": np.ascontiguousarray(bq[sl]),
            "gk_s": np.ascontiguousarray(gk[sl]),
            "bk_s": np.ascontiguousarray(bk[sl]),
        })

    _CACHE["in_maps"] = in_maps
    try:
        res = run_bass_kernel_spmd(nc, in_maps, list(range(NCORES))).results
    except Exception:
        # transient runtime failures (device wedged from a prior run) --
        # one retry typically succeeds
        res = run_bass_kernel_spmd(nc, in_maps, list(range(NCORES))).results
    y = np.empty((B, NQ, D), dtype=np.float32)
    for c in range(NCORES):
        b, r = c // TEAM, c % TEAM
        y[b, :, DSL * r:DSL * (r + 1)] = res[c]["y_col"]
    return y
